# revision 11
# baseline (speedup 1.0000x reference)
"""KPConv aggregate layer on 8 trn2 NeuronCores.

Math (per batch b):
    sq_d[n,k]  = ||p[n] - kp[k]||^2
    aw[n,k]    = relu(1 - sqrt(sq_d)/KP_EXTENT)
    wf[k,c]    = sum_n aw[n,k] * x[c,n]
    out[o]     = sum_{k,c} wf[k,c] * W[k,c,o]

Sharding: data-parallel over B=8 across the 8 cores (batch b -> core b).
Per core the kernel streams x (32 MB) once from HBM (memory roofline),
computes aw on DVE/ACT from PE-transposed point coords, transposes x
tiles on the PE (fp16) and accumulates wf with 15-wide stationary
matmuls into PSUM, then applies the tiny [15,128,128] GEMM.
"""

import numpy as np
from contextlib import ExitStack

import concourse.bass as bass
import concourse.mybir as mybir
import concourse.tile as tile
from concourse import bacc
from concourse.bass_utils import run_bass_kernel_spmd

B, N, C, K = 8, 65536, 128, 15
KP_EXTENT = 1.0 * 1.2 / 2.5  # 0.48
NCH = N // 128        # 512 chunks of 128 points
NI = NCH // 4         # 128 chunk-columns per q-group
KW = K * NI           # 1920 columns of the aw / kxb tiles
NSLICE = 4            # sq_d pipeline slices per q-group (pipelining)
XT = 2048             # x DMA tile free size
NXT = N // XT         # 32 x tiles

f32 = mybir.dt.float32
f16 = mybir.dt.float16


def _ap3(t, off_elems, pdim, d1, d2):
    """Build a 3-D access pattern [pdim, d1, d2] over tile ap `t`."""
    return bass.AP(t.tensor, t.offset + off_elems, [t.ap[0][:], list(d1), list(d2)])


def build_nc():
    nc = bacc.Bacc("TRN2", target_bir_lowering=False, debug=False, num_devices=B)

    x_d = nc.dram_tensor("x", [C, N], f32, kind="ExternalInput")
    pp_d = nc.dram_tensor("pp", [128, 1536], f32, kind="ExternalInput")
    kxb_d = nc.dram_tensor("kxb", [128, KW], f16, kind="ExternalInput")
    kyb_d = nc.dram_tensor("kyb", [128, KW], f16, kind="ExternalInput")
    kzb_d = nc.dram_tensor("kzb", [128, KW], f16, kind="ExternalInput")
    eye16_d = nc.dram_tensor("eye16", [128, 128], f16, kind="ExternalInput")
    eye32_d = nc.dram_tensor("eye32", [128, 128], f32, kind="ExternalInput")
    wsb_d = nc.dram_tensor("wsb", [C, K * 128], f32, kind="ExternalInput")
    out_d = nc.dram_tensor("out", [1, 128], f32, kind="ExternalOutput")

    with tile.TileContext(nc) as tc, ExitStack() as ctx:
        consts = ctx.enter_context(tc.tile_pool(name="consts", bufs=1))
        ppool = ctx.enter_context(tc.tile_pool(name="ppool", bufs=1))
        awpool = ctx.enter_context(tc.tile_pool(name="awpool", bufs=1))
        tmp = ctx.enter_context(tc.tile_pool(name="tmp", bufs=3))
        xpool = ctx.enter_context(tc.tile_pool(name="xpool", bufs=4))
        xhpool = ctx.enter_context(tc.tile_pool(name="xhpool", bufs=4))
        xspool = ctx.enter_context(tc.tile_pool(name="xspool", bufs=12))
        ps_t = ctx.enter_context(tc.tile_pool(name="ps_t", bufs=2, space="PSUM"))
        ps_x = ctx.enter_context(tc.tile_pool(name="ps_x", bufs=4, space="PSUM"))
        ps_wf = ctx.enter_context(tc.tile_pool(name="ps_wf", bufs=1, space="PSUM"))
        fin = ctx.enter_context(tc.tile_pool(name="fin", bufs=1))

        # ---- constants / setup ------------------------------------------
        eye16 = consts.tile([128, 128], f16)
        nc.sync.dma_start(eye16, eye16_d.ap())
        eye32 = consts.tile([128, 128], f32)
        nc.sync.dma_start(eye32, eye32_d.ap())
        kxb = consts.tile([128, KW], f16)
        nc.sync.dma_start(kxb, kxb_d.ap())
        kyb = consts.tile([128, KW], f16)
        nc.sync.dma_start(kyb, kyb_d.ap())
        kzb = consts.tile([128, KW], f16)
        nc.sync.dma_start(kzb, kzb_d.ap())
        wsb = consts.tile([C, K * 128], f32)
        nc.sync.dma_start(wsb, wsb_d.ap())

        pp = ppool.tile([128, 1536], f32)
        nc.sync.dma_start(pp, pp_d.ap())

        # deinterleave xyz:  pc[d][g, j] = coord d of point n = 512*g + j
        pcs = []
        for d in range(3):
            pc = ppool.tile([128, 512], f32, name=f"pc{d}")
            src = bass.AP(pp.tensor, pp.offset + d, [pp.ap[0][:], [3, 512]])
            nc.vector.tensor_copy(pc, src)
            pcs.append(pc)

        # PE-transpose to [j', chunk-col] layout (fp16):
        # P[d][q][j, i] = coord d of point n = 512*i + 128*q + j
        P = [[None] * 4 for _ in range(3)]
        for d in range(3):
            for q in range(4):
                pt = ps_t.tile([128, 128], f32, name=f"pt{d}{q}", tag="pt")
                nc.tensor.transpose(pt, pcs[d][:, 128 * q:128 * (q + 1)], eye32)
                pq = ppool.tile([128, 128], f16, name=f"p{d}{q}")
                nc.vector.tensor_copy(pq, pt)
                P[d][q] = pq

        # ---- aw pipeline: aw[q][j, 128k+i] ------------------------------
        AW = []
        for q in range(4):
            aw = awpool.tile([128, KW], f16, name=f"aw{q}")
            AW.append(aw)
        ksrc = [kxb, kyb, kzb]
        for q in range(4):
            for s in range(NSLICE):
                il = NI // NSLICE
                i0 = s * il
                acc = None
                for d in range(3):
                    dx = tmp.tile([128, K * il], f16, tag="dx", name=f"dx{q}{s}{d}")
                    dx3 = _ap3(dx, 0, None, [il, K], [1, il])
                    pb = _ap3(P[d][q], i0, None, [0, K], [1, il])
                    kb = _ap3(ksrc[d], i0, None, [NI, K], [1, il])
                    nc.vector.tensor_tensor(
                        dx3, pb, kb, op=mybir.AluOpType.subtract)
                    sx = tmp.tile([128, K * il], f16, tag="sx", name=f"sx{q}{s}{d}")
                    nc.vector.tensor_tensor(
                        sx, dx, dx, op=mybir.AluOpType.mult)
                    if acc is None:
                        acc = sx
                    else:
                        a2 = tmp.tile([128, K * il], f16, tag="acc",
                                      name=f"acc{q}{s}{d}")
                        nc.vector.tensor_tensor(
                            a2, acc, sx, op=mybir.AluOpType.add)
                        acc = a2
                rt = tmp.tile([128, K * il], f16, tag="rt", name=f"rt{q}{s}")
                nc.scalar.sqrt(rt, acc)
                awsl = _ap3(AW[q], i0, None, [NI, K], [1, il])
                nc.scalar.activation(
                    awsl, rt, mybir.ActivationFunctionType.Relu,
                    bias=1.0, scale=-1.0 / KP_EXTENT)

        # ---- main x loop -------------------------------------------------
        wf = ps_wf.tile([K, 128], f32)
        for j in range(NXT):
            xt = xpool.tile([128, XT], f32, tag="xt")
            nc.sync.dma_start(xt, x_d.ap()[:, XT * j:XT * (j + 1)])
            xh = xhpool.tile([128, XT], f16, tag="xh")
            nc.scalar.copy(xh, xt)
            for h in range(2):
                ps = ps_x.tile([128, 1024], f16, tag="psx", name=f"psx{j}{h}")
                for u in range(8):
                    nc.tensor.transpose(
                        ps[:, 128 * u:128 * (u + 1)],
                        xh[:, 1024 * h + 128 * u:1024 * h + 128 * (u + 1)],
                        eye16)
                xs = xspool.tile([128, 1024], f16, tag="xs")
                nc.vector.tensor_copy(xs, ps)
                for u in range(8):
                    m = 16 * j + 8 * h + u
                    i, q = m // 4, m % 4
                    lhsT = bass.AP(AW[q].tensor, AW[q].offset + i,
                                   [AW[q].ap[0][:], [NI, K]])
                    nc.tensor.matmul(
                        wf, lhsT, xs[:, 128 * u:128 * (u + 1)],
                        start=(m == 0), stop=(m == NCH - 1),
                        skip_group_check=True)

        # ---- stage 2: out[o] = sum_k wf[k,:] @ W[k] ----------------------
        wf_sb = fin.tile([K, 128], f32)
        nc.vector.tensor_copy(wf_sb, wf)
        wft_ps = ps_t.tile([128, K], f32, tag="pt")
        nc.tensor.transpose(wft_ps, wf_sb, eye32[:K, :K])
        wft = fin.tile([128, K], f32)
        nc.vector.tensor_copy(wft, wft_ps)
        o_ps = ps_t.tile([1, 128], f32, tag="pt")
        for k in range(K):
            nc.tensor.matmul(
                o_ps, wft[:, k:k + 1], wsb[:, 128 * k:128 * (k + 1)],
                start=(k == 0), stop=(k == K - 1), skip_group_check=True)
        o_sb = fin.tile([1, 128], f32)
        nc.vector.tensor_copy(o_sb, o_ps)
        nc.sync.dma_start(out_d.ap(), o_sb)

    nc.compile()
    return nc


def make_inputs(p, x, weights, kernel_points):
    p = np.asarray(p, np.float32)
    x = np.ascontiguousarray(np.asarray(x, np.float32))
    w = np.asarray(weights, np.float32)
    kp = np.asarray(kernel_points, np.float32)

    kb = [np.ascontiguousarray(
        np.broadcast_to(np.repeat(kp[:, d], NI)[None, :], (128, KW))
    ).astype(np.float16) for d in range(3)]
    eye16 = np.eye(128, dtype=np.float16)
    eye32 = np.eye(128, dtype=np.float32)
    wsb = np.ascontiguousarray(w.transpose(1, 0, 2).reshape(C, K * 128))

    in_maps = []
    for b in range(B):
        in_maps.append({
            "x": np.ascontiguousarray(x[b]),
            "pp": np.ascontiguousarray(p[b].reshape(128, 1536)),
            "kxb": kb[0], "kyb": kb[1], "kzb": kb[2],
            "eye16": eye16, "eye32": eye32, "wsb": wsb,
        })
    return in_maps


_NC_CACHE = None


def _get_nc():
    global _NC_CACHE
    if _NC_CACHE is None:
        _NC_CACHE = build_nc()
    return _NC_CACHE


def kernel(p, x, weights, kernel_points):
    nc = _get_nc()
    in_maps = make_inputs(p, x, weights, kernel_points)
    res = run_bass_kernel_spmd(nc, in_maps, core_ids=list(range(B)))
    out = np.concatenate([res.results[b]["out"] for b in range(B)], axis=0)
    return out.astype(np.float32)
